# revision 12
# baseline (speedup 1.0000x reference)
"""Trainium2 Bass kernel for nn_LoraAttention.

Math (reference): qkv = x@W_qkv.T; lora full proj ql/vl = split(x@W_lora.T + b_lora)
(K-part discarded); low-rank dq = (x@A_q.T)@B_q.T*1/8 (same for v); softmax
attention over H=16 heads, D=64; out = attn_cat@W_out.T + b_out.

Host-side algebra folds every LoRA term into the projection weights:
  Wq_eff = W_qkv[q] + W_lora[q] + (B_q@A_q)/8      (q bias b_lora[q] kept)
  Wk_eff = W_qkv[k]                                 (no bias)
  Wv_eff = W_qkv[v] + W_lora[v] + (B_v@A_v)/8
  v bias b_lora[v] commutes through softmax -> folded into host-side output
  bias: b_eff = b_out + W_out @ b_lora[v].

Sharding: 8 cores = 4 batches x 2 head-groups (8 heads each).  Each core
projects QKV for its heads, does attention, and computes a partial output
projection over its 512 concat dims; host sums the two partials per batch
(partials shipped bf16, summed fp32 on host).

Device dataflow per core (matmuls bf16 in / fp32 accum):
  - warm-up: ~24 dummy matmuls on memset tiles issue from t~0 so the PE HAM
    clock-gate reaches 8/8 (2.4 GHz) before real data lands.
  - DMA order is criticality-sorted and split across two queues (sync: x
    token-half 0, wv, x token-half 1; gpsimd: wqk in 256-col chunks with the
    pair-0/1 K and Q chunks first, then wo) so the first S^T can issue ~17us.
  - pair-0's V projections and remaining K/Q chains drain one-per-mq-slot
    inside pair 0's attention windows instead of running as a monolithic
    block that starves ScalarE.
  - S^T = K^T@Q per head via row-packed (tile_position) pairs of K=64
    matmuls; exp on ScalarE from PSUM (scale=1/8, bf16 out); P@[V|1] matmuls
    put raw attention in rows 0..63 and the softmax denominator in row 64.
  - a subset of key-chunks (3 of 16 per window, skipping pair0-nq0) computes
    exp on the DVE instead via the Schraudolph bit trick: one tensor_scalar
    (s*23.083+16250.5 -> int16, bitcast bf16) approximates exp(s/8) to ~3%;
    the constant offset cancels in softmax normalization.  This moves ~18%
    of the exp rail off the saturated ScalarE.
  - normalization per (pair, nq), software-pipelined by one nq so nothing
    stalls the in-order PE queue (deferred-stage machinery as before).
  - output projection for chunk nq emitted inside pair 3 right after that
    nq's normalization; partials evicted bf16 and DMA'd from the gpsimd
    queue.
"""

import numpy as np
import ml_dtypes

import concourse.bacc as bacc
import concourse.tile as tile
from concourse import mybir
from concourse.bass_utils import run_bass_kernel_spmd

B, N, C = 4, 2048, 1024
H, D = 16, 64
LORA_SCALE = 1.0 / 8.0
ATTN_SCALE = float(D) ** -0.5  # 0.125

f32 = mybir.dt.float32
bf16 = mybir.dt.bfloat16
i16 = mybir.dt.int16
BF = ml_dtypes.bfloat16

NQ = 4           # token chunks of 512 for moving operands
MQ = 16          # key/token chunks of 128 for S^T partition dim
KC = 8           # contraction chunks of 128 over C
PAIRS = 4        # head pairs per core (8 local heads)

N_WARM = 24      # PE warm-up dummy matmuls

# Schraudolph exp-approx constants: exp(s*0.125) ~= bitcast_bf16(int16(
#   s * (0.125*128/ln2) + (16256 - 128*0.043) )).  The -0.043 shift centres
# the piecewise-linear 2^frac error; any constant offset cancels in softmax.
SCHR_SCALE = 0.125 * 128.0 / float(np.log(2.0))
SCHR_MAGIC = 16256.0 - 128.0 * 0.043
# Per-pair sets of mq slots whose exp runs SPLIT: one head's half on ScalarE
# (exact), the other on DVE (Schraudolph).  Splitting halves the sp-buffer
# hold time per slot, which is what paces the S^T ring; pair 0 is PE-bound
# (chain deficit) and pair 3 carries the out-projection, so they split less.
SPLIT_MQS = {
    0: (),
    1: (1, 2, 4, 5, 7, 8, 10, 11, 13, 14),
    2: (1, 2, 4, 5, 7, 8, 10, 11, 13, 14),
    3: (2, 5, 8, 11, 14),
}

_cache: dict = {}


def _build_program():
    nc = bacc.Bacc("TRN2", target_bir_lowering=False, debug=False, num_devices=8)

    xT_d = nc.dram_tensor("xT", [C, N], bf16, kind="ExternalInput").ap()
    wqk_d = nc.dram_tensor("wqk", [C, 1024], bf16, kind="ExternalInput").ap()
    wv_d = nc.dram_tensor("wv", [C, 512], bf16, kind="ExternalInput").ap()
    wo_d = nc.dram_tensor("wo", [512, C], bf16, kind="ExternalInput").ap()
    bq_d = nc.dram_tensor("bq", [128, 4], f32, kind="ExternalInput").ap()
    outT_d = nc.dram_tensor("outT", [C, N], bf16, kind="ExternalOutput").ap()

    EXP = mybir.ActivationFunctionType.Exp
    MULT = mybir.AluOpType.mult
    ADD = mybir.AluOpType.add

    with tile.TileContext(nc) as tc:
        with (
            tc.tile_pool(name="win", bufs=1) as win,        # weights + x + consts
            tc.tile_pool(name="kq", bufs=1) as kqp,         # K/Q bf16 tiles
            tc.tile_pool(name="vp", bufs=1) as vp,          # [V|1] tiles
            tc.tile_pool(name="pex", bufs=6) as pex,        # exp outputs
            tc.tile_pool(name="acat", bufs=1) as acatp,     # normalized attn (d, nq)
            tc.tile_pool(name="scr", bufs=1) as scr,        # norm scratch
            tc.tile_pool(name="osb", bufs=6) as osbp,       # out eviction
            tc.tile_pool(name="ps", bufs=1, space="PSUM") as psp,
        ):
            # ---- warm-up inputs (no DMA deps; memsets on vector).  Dummies
            # use the full 128x128 array so the HAM activity monitor counts
            # them and un-throttles the PE clock before real data lands. ----
            dumw = win.tile([128, 128], bf16, tag="dumw")
            dumi = win.tile([128, 512], bf16, tag="dumi")
            nc.vector.memset(dumw[:], 0.0)
            nc.vector.memset(dumi[:], 0.0)
            # ones row at partition 64 for the denominator-broadcast matmul,
            # which contracts at tile_position (64,0) straight from the
            # reciprocal computed in-lane at partition 64
            ones65 = win.tile([65, 64], bf16, tag="ones65")
            nc.vector.memset(ones65[64:65, :], 1.0)

            # ---- loads, criticality-sorted across two queues ----
            # sync queue: bq, x token-half 0, wv, x token-half 1
            # gpsimd queue: wqk 256-col chunks (K/Q for pairs 0-1 first), wo
            bqt = win.tile([128, 4], f32, tag="bq")
            nc.sync.dma_start(bqt[:], bq_d[:])
            xt, wqk, wv = [], [], []
            for kc in range(KC):
                t = win.tile([128, N], bf16, tag=f"xt{kc}")
                nc.sync.dma_start(
                    t[:, 0:1024], xT_d[kc * 128:(kc + 1) * 128, 0:1024]
                )
                xt.append(t)
                t = win.tile([128, 1024], bf16, tag=f"wqk{kc}")
                wqk.append(t)
                t = win.tile([128, 512], bf16, tag=f"wv{kc}")
                wv.append(t)
            # K chunks for pairs 0-1 ([512:768]) then Q chunks pairs 0-1
            for lo in (512, 0):
                for kc in range(KC):
                    nc.gpsimd.dma_start(
                        wqk[kc][:, lo:lo + 256],
                        wqk_d[kc * 128:(kc + 1) * 128, lo:lo + 256],
                    )
            for kc in range(KC):
                nc.sync.dma_start(wv[kc][:], wv_d[kc * 128:(kc + 1) * 128, :])
            for kc in range(KC):
                nc.sync.dma_start(
                    xt[kc][:, 1024:2048], xT_d[kc * 128:(kc + 1) * 128, 1024:2048]
                )
            # K/Q chunks for pairs 2-3
            for lo in (768, 256):
                for kc in range(KC):
                    nc.gpsimd.dma_start(
                        wqk[kc][:, lo:lo + 256],
                        wqk_d[kc * 128:(kc + 1) * 128, lo:lo + 256],
                    )
            wo = []
            for dc in range(4):
                t = win.tile([128, 1024], bf16, tag=f"wo{dc}")
                nc.gpsimd.dma_start(t[:], wo_d[dc * 128:(dc + 1) * 128, :])
                wo.append(t)

            acat = [[None] * PAIRS for _ in range(NQ)]

            # shared pp/rb tag alternator: consecutive projection chains
            # land in different banks so eviction overlaps the next chain
            ps_flip = [0]

            def proj_ps():
                ps_flip[0] ^= 1
                return psp.tile(
                    [128, 512], f32, name="ps",
                    tag=("pp" if ps_flip[0] else "rb"),
                )

            # ---- PE warm-up: dummies into the pp bank, serial, data-free ----
            warm_ps = psp.tile([128, 512], f32, name="ps", tag="pp")
            for _ in range(N_WARM):
                nc.tensor.matmul(
                    warm_ps[:], dumw[:], dumi[:],
                    start=True, stop=True,
                )

            def kq_group(t, kt, qt, j):
                kind, nq = divmod(j, NQ)
                ps = proj_ps()
                off = (512 if kind == 0 else 0) + t * 128
                for kc in range(KC):
                    nc.tensor.matmul(
                        ps[:],
                        wqk[kc][:, off:off + 128],
                        xt[kc][:, nq * 512:(nq + 1) * 512],
                        start=(kc == 0), stop=(kc == KC - 1),
                    )
                if kind == 0:
                    nc.vector.tensor_copy(kt[:, nq * 512:(nq + 1) * 512], ps[:])
                else:
                    nc.vector.tensor_scalar_add(
                        qt[:, nq * 512:(nq + 1) * 512], ps[:], bqt[:, t:t + 1]
                    )

            def v_proj(mq):
                vt = vp.tile([128, 8, 65], bf16, tag=f"v{mq}")
                nc.vector.memset(vt[:, :, 64:65], 1.0)
                ps = proj_ps()
                for kc in range(KC):
                    nc.tensor.matmul(
                        ps[:], xt[kc][:, mq * 128:(mq + 1) * 128], wv[kc][:],
                        start=(kc == 0), stop=(kc == KC - 1),
                    )
                nc.vector.tensor_copy(
                    vt[:, :, 0:64], ps[:].rearrange("p (h e) -> p h e", h=8)
                )
                return vt

            vts = [None] * MQ

            def v_chain(mq):
                def go():
                    vts[mq] = v_proj(mq)
                return go

            J_ORDER = [0, 4, 1, 5, 2, 6, 3, 7]

            def out_proj_cc(nq, cc):
                ps = proj_ps()
                for dc in range(4):
                    nc.tensor.matmul(
                        ps[:],
                        wo[dc][:, cc * 128:(cc + 1) * 128],
                        acat[nq][dc][:],
                        start=(dc == 0), stop=(dc == 3),
                    )
                ob = osbp.tile([128, 512], bf16, tag="ob")
                nc.vector.tensor_copy(ob[:], ps[:])
                nc.gpsimd.dma_start(
                    outT_d[cc * 128:(cc + 1) * 128, nq * 512:(nq + 1) * 512],
                    ob[:],
                )

            # Deferred normalization: stage 0 (evict + reciprocal) runs at the
            # nq seam; the remaining stages are emitted one-per-mq inside the
            # next nq's loop so their PE work never waits on DVE round-trips.
            def make_norm(t, nq, atA, atB):
                ac = acatp.tile([128, 512], bf16, tag=f"ac{nq}_{t}")
                acat[nq][t] = ac
                st = {}

                def stage0():
                    sA = scr.tile([64, 512], f32, tag="sA")
                    sB = scr.tile([64, 512], f32, tag="sB")
                    r32A = scr.tile([65, 512], f32, tag="r32A")
                    r32B = scr.tile([65, 512], f32, tag="r32B")
                    # jump the DVE queue: the reciprocals start the den
                    # critical path and the evictions gate the next nq's
                    # first PV (PSUM WAR) and with it the whole PE pipe
                    with tc.high_priority():
                        nc.vector.reciprocal_approx_fast(
                            r32A[64:65, :], atA[64:65, :]
                        )
                        nc.vector.reciprocal_approx_fast(
                            r32B[64:65, :], atB[64:65, :]
                        )
                        nc.vector.tensor_copy(sA[:], atA[0:64, :])
                        nc.vector.tensor_copy(sB[:], atB[0:64, :])
                    rrA = scr.tile([65, 512], bf16, tag="rrA")
                    rrB = scr.tile([65, 512], bf16, tag="rrB")
                    nc.vector.tensor_copy(rrA[64:65, :], r32A[64:65, :])
                    nc.vector.tensor_copy(rrB[64:65, :], r32B[64:65, :])
                    st.update(sA=sA, sB=sB, rrA=rrA, rrB=rrB)

                def half(which):
                    s, rr = (st["sA"], st["rrA"]) if which == 0 else (st["sB"], st["rrB"])
                    rb = psp.tile([64, 512], f32, tag="rb")
                    nc.tensor.matmul(
                        rb[:], ones65[64:65, :], rr[64:65, :],
                        start=True, stop=True, tile_position=(64, 0),
                    )
                    if which == 0:
                        nc.vector.tensor_mul(ac[0:64, :], s[0:64, :], rb[:])
                    else:
                        acn = scr.tile([64, 512], bf16, tag="acn")
                        nc.vector.tensor_mul(acn[:], s[0:64, :], rb[:])
                        nc.sync.dma_start(ac[64:128, :], acn[:])

                return [stage0, lambda: half(0), lambda: half(1)] + (
                    [lambda cc=cc: out_proj_cc(nq, cc) for cc in range(8)]
                    if t == PAIRS - 1 else []
                )

            pending: list = []
            deferred: list = []   # pair-0 local chains, drained 1/mq-slot
            prefetch: list = []   # next-pair chains, drained every few slots

            # ---- pair 0 minimal pre-loop: K(nq0), Q(nq0), V(0..3) ----
            kt0 = kqp.tile([128, N], bf16, tag="k0")
            qt0 = kqp.tile([128, N], bf16, tag="q0")
            kq_group(0, kt0, qt0, 0)
            kq_group(0, kt0, qt0, 4)
            for mq in range(4):
                vts[mq] = v_proj(mq)
            # remaining pair-0 work, deadline-ordered for the mq-slot drain
            deferred += [lambda: kq_group(0, kt0, qt0, 1)]
            deferred += [v_chain(m) for m in (4, 5, 6)]
            deferred += [lambda: kq_group(0, kt0, qt0, 2)]
            deferred += [v_chain(m) for m in (7, 8, 9)]
            deferred += [lambda: kq_group(0, kt0, qt0, 3)]
            deferred += [v_chain(m) for m in (10, 11, 12)]
            deferred += [lambda: kq_group(0, kt0, qt0, 5)]
            deferred += [v_chain(m) for m in (13, 14, 15)]
            deferred += [
                lambda: kq_group(0, kt0, qt0, 6),
                lambda: kq_group(0, kt0, qt0, 7),
            ]
            kq_tiles = {0: (kt0, qt0)}

            # ---- per pair: attention; drain deferred chains; pipelined norm ----
            for t in range(PAIRS):
                kt, qt = kq_tiles.pop(t)
                if t + 1 < PAIRS:
                    ktn = kqp.tile([128, N], bf16, tag=f"k{(t + 1) % 2}")
                    qtn = kqp.tile([128, N], bf16, tag=f"q{(t + 1) % 2}")
                    kq_tiles[t + 1] = (ktn, qtn)
                    for j in J_ORDER:
                        prefetch.append(
                            lambda t1=t + 1, kt1=ktn, qt1=qtn, j1=j:
                            kq_group(t1, kt1, qt1, j1)
                        )
                # next-pair chains spread over this pair's ScalarE-bound
                # windows: pair 0 over nq 1-3, others over all 4
                nq_lo, step = (1, 5) if t == 0 else (0, 8)
                for nq in range(NQ):
                    atA = psp.tile([65, 512], f32, tag="atA")
                    atB = psp.tile([65, 512], f32, tag="atB")
                    for mq in range(MQ):
                        if deferred:
                            deferred.pop(0)()
                        elif prefetch and nq >= nq_lo:
                            fi = (nq - nq_lo) * MQ + mq
                            if fi % step == 0:
                                prefetch.pop(0)()
                        sp = psp.tile([128, 1024], f32, tag="sp", bufs=2)
                        # highest priority: S results feed ScalarE, the
                        # second rail — never let exp starve behind proj work
                        with tc.high_priority():
                            nc.tensor.matmul(
                                sp[:, 0:512],
                                kt[0:64, mq * 128:(mq + 1) * 128],
                                qt[0:64, nq * 512:(nq + 1) * 512],
                                start=True, stop=True, tile_position=(0, 0),
                            )
                            nc.tensor.matmul(
                                sp[:, 512:1024],
                                kt[64:128, mq * 128:(mq + 1) * 128],
                                qt[64:128, nq * 512:(nq + 1) * 512],
                                start=True, stop=True, tile_position=(64, 0),
                            )
                        # drain one stage of the deferred norm pipeline per mq
                        if pending and 1 <= mq <= len(pending):
                            pending[mq - 1]()
                            if mq == len(pending):
                                pending.clear()
                        # deep pe ring: ACT(k) only WARs on PV(k-12), so the
                        # exp stream never waits on V-projection availability
                        # in (0,0) and rides over every nq seam
                        pe = pex.tile([128, 1024], bf16, tag="pe", bufs=16)
                        if mq in SPLIT_MQS[t]:
                            # split exp: exact half on ScalarE, Schraudolph
                            # half on DVE, concurrently — halves the time the
                            # sp buffer is held, which paces the S^T ring.
                            # Alternate which head is approximated.
                            ex, ap = (0, 512) if mq % 2 else (512, 0)
                            nc.scalar.activation(
                                pe[:, ex:ex + 512], sp[:, ex:ex + 512],
                                EXP, scale=ATTN_SCALE,
                            )
                            nc.vector.tensor_scalar(
                                pe[:, ap:ap + 512].bitcast(i16),
                                sp[:, ap:ap + 512],
                                SCHR_SCALE, SCHR_MAGIC, MULT, ADD,
                            )
                        else:
                            nc.scalar.activation(pe[:], sp[:], EXP, scale=ATTN_SCALE)
                        nc.tensor.matmul(
                            atA[:], vts[mq][:, 2 * t, :], pe[:, 0:512],
                            start=(mq == 0), stop=(mq == MQ - 1),
                        )
                        nc.tensor.matmul(
                            atB[:], vts[mq][:, 2 * t + 1, :], pe[:, 512:1024],
                            start=(mq == 0), stop=(mq == MQ - 1),
                        )
                    stages = make_norm(t, nq, atA, atB)
                    # evict + reciprocal immediately (clears PSUM); defer rest
                    stages[0]()
                    pending = stages[1:]
            # tail warm-keepers: cover the final norm latency so the HAM
            # clock-gate stays at 8/8 for the last out-projection chains
            for _ in range(12):
                wps = psp.tile([65, 512], f32, name="ps", tag="atB")
                nc.tensor.matmul(
                    wps[:], dumw[:, 0:65], dumi[:], start=True, stop=True,
                )
            # drain the last norm (pair 3, nq 3) and its out projection
            for s in pending:
                s()
            pending = []

    nc.compile()
    return nc


def _get_program():
    if "nc" not in _cache:
        _cache["nc"] = _build_program()
    return _cache["nc"]


def _prep_in_maps(x, W_qkv, W_lora, b_lora, A_q, B_q, A_v, B_v, W_out):
    HD = H * D  # 1024
    Wq = W_qkv[0:HD] + W_lora[0:HD] + LORA_SCALE * (B_q @ A_q)
    Wk = W_qkv[HD:2 * HD]
    Wv = W_qkv[2 * HD:3 * HD] + W_lora[2 * HD:3 * HD] + LORA_SCALE * (B_v @ A_v)
    bq = b_lora[0:HD]

    xT = [np.ascontiguousarray(x[b].T).astype(BF) for b in range(B)]
    in_maps = []
    for c in range(8):
        b, hg = divmod(c, 2)
        sel = slice(hg * 512, (hg + 1) * 512)
        wqk_c = np.ascontiguousarray(
            np.concatenate([Wq[sel], Wk[sel]], axis=0).T
        ).astype(BF)
        wv_c = np.ascontiguousarray(Wv[sel].T).astype(BF)
        wo_c = np.ascontiguousarray(W_out[:, sel].T).astype(BF)
        bq_c = np.ascontiguousarray(bq[sel].reshape(4, 128).T).astype(np.float32)
        in_maps.append({
            "xT": xT[b], "wqk": wqk_c, "wv": wv_c, "wo": wo_c, "bq": bq_c,
        })
    return in_maps


def kernel(x, W_qkv, W_lora, b_lora, A_q, B_q, A_v, B_v, W_out, b_out):
    x = np.asarray(x, np.float32)
    W_qkv = np.asarray(W_qkv, np.float32)
    W_lora = np.asarray(W_lora, np.float32)
    b_lora = np.asarray(b_lora, np.float32)
    A_q = np.asarray(A_q, np.float32)
    B_q = np.asarray(B_q, np.float32)
    A_v = np.asarray(A_v, np.float32)
    B_v = np.asarray(B_v, np.float32)
    W_out = np.asarray(W_out, np.float32)
    b_out = np.asarray(b_out, np.float32)

    in_maps = _prep_in_maps(x, W_qkv, W_lora, b_lora, A_q, B_q, A_v, B_v, W_out)
    b_eff = b_out + W_out @ b_lora[2 * H * D:3 * H * D]

    nc = _get_program()
    res = run_bass_kernel_spmd(nc, in_maps, list(range(8)))

    out = np.empty((B, N, C), np.float32)
    for b in range(B):
        acc = res.results[2 * b]["outT"].astype(np.float32)
        acc += res.results[2 * b + 1]["outT"].astype(np.float32)
        acc += b_eff[:, None]
        out[b] = acc.T
    return out


# revision 24
# speedup vs baseline: 1.0617x; 1.0617x over previous
"""Trainium2 Bass kernel for nn_LoraAttention.

Math (reference): qkv = x@W_qkv.T; lora full proj ql/vl = split(x@W_lora.T + b_lora)
(K-part discarded); low-rank dq = (x@A_q.T)@B_q.T*1/8 (same for v); softmax
attention over H=16 heads, D=64; out = attn_cat@W_out.T + b_out.

Host-side algebra folds every LoRA term into the projection weights:
  Wq_eff = W_qkv[q] + W_lora[q] + (B_q@A_q)/8      (q bias b_lora[q] kept)
  Wk_eff = W_qkv[k]                                 (no bias)
  Wv_eff = W_qkv[v] + W_lora[v] + (B_v@A_v)/8
  v bias b_lora[v] commutes through softmax -> folded into host-side output
  bias: b_eff = b_out + W_out @ b_lora[v].

Sharding: 8 cores = 4 batches x 2 head-groups (8 heads each).  Each core
projects QKV for its heads, does attention, and computes a partial output
projection over its 512 concat dims; host sums the two partials per batch
(partials shipped bf16, summed fp32 on host).

Device dataflow per core (matmuls bf16 in / fp32 accum):
  - warm-up: ~24 dummy matmuls on memset tiles issue from t~0 so the PE HAM
    clock-gate reaches 8/8 (2.4 GHz) before real data lands.
  - DMA order is criticality-sorted and split across two queues (sync: x
    token-half 0, wv, x token-half 1; gpsimd: wqk in 256-col chunks with the
    pair-0/1 K and Q chunks first, then wo) so the first S^T can issue ~17us.
  - pair-0's V projections and remaining K/Q chains drain one-per-mq-slot
    inside pair 0's attention windows instead of running as a monolithic
    block that starves ScalarE.
  - S^T = K^T@Q per head via row-packed (tile_position) pairs of K=64
    matmuls; exp on ScalarE from PSUM (scale=1/8, bf16 out); P@[V|1] matmuls
    put raw attention in rows 0..63 and the softmax denominator in row 64.
  - a subset of key-chunks (3 of 16 per window, skipping pair0-nq0) computes
    exp on the DVE instead via the Schraudolph bit trick: one tensor_scalar
    (s*23.083+16250.5 -> int16, bitcast bf16) approximates exp(s/8) to ~3%;
    the constant offset cancels in softmax normalization.  This moves ~18%
    of the exp rail off the saturated ScalarE.
  - normalization per (pair, nq), software-pipelined by one nq so nothing
    stalls the in-order PE queue (deferred-stage machinery as before).
  - output projection for chunk nq emitted inside pair 3 right after that
    nq's normalization; partials evicted bf16 and DMA'd from the gpsimd
    queue.
"""

import numpy as np
import ml_dtypes

import concourse.bacc as bacc
import concourse.tile as tile
from concourse import mybir
from concourse.bass_utils import run_bass_kernel_spmd

B, N, C = 4, 2048, 1024
H, D = 16, 64
LORA_SCALE = 1.0 / 8.0
ATTN_SCALE = float(D) ** -0.5  # 0.125

f32 = mybir.dt.float32
bf16 = mybir.dt.bfloat16
i16 = mybir.dt.int16
BF = ml_dtypes.bfloat16

NQ = 4           # token chunks of 512 for moving operands
MQ = 16          # key/token chunks of 128 for S^T partition dim
KC = 8           # contraction chunks of 128 over C
PAIRS = 4        # head pairs per core (8 local heads)

N_WARM = 44      # PE warm-up dummy matmuls (bridge until DMA-fed chains start)

# Schraudolph exp-approx constants: exp(s*0.125) ~= bitcast_bf16(int16(
#   s * (0.125*128/ln2) + (16256 - 128*0.043) )).  The -0.043 shift centres
# the piecewise-linear 2^frac error; any constant offset cancels in softmax.
SCHR_SCALE = 0.125 * 128.0 / float(np.log(2.0))
SCHR_MAGIC = 16256.0 - 128.0 * 0.043
# Per-pair sets of mq slots whose exp tile is computed on the DVE instead of
# ScalarE.  Few per window: the sp ring couples S^T(k+2) to reader(k), and
# the DVE (shared with evictions/norm work) serves reads less promptly than
# the dedicated ACT queue — more offload measurably slows the ring.  Pair 0
# is PE-bound (chain deficit) and pair 3's DVE carries out-proj evictions.
SCHR_MQS = {
    0: (),
    1: (4, 9, 14),
    2: (4, 9, 14),
    3: (9,),
}

_cache: dict = {}


def _build_program():
    nc = bacc.Bacc("TRN2", target_bir_lowering=False, debug=False, num_devices=8)

    xT_d = nc.dram_tensor("xT", [C, N], bf16, kind="ExternalInput").ap()
    wqk_d = nc.dram_tensor("wqk", [C, 1024], bf16, kind="ExternalInput").ap()
    wv_d = nc.dram_tensor("wv", [C, 512], bf16, kind="ExternalInput").ap()
    wo_d = nc.dram_tensor("wo", [512, C], bf16, kind="ExternalInput").ap()
    bq_d = nc.dram_tensor("bq", [128, 4], f32, kind="ExternalInput").ap()
    outT_d = nc.dram_tensor("outT", [C, N], bf16, kind="ExternalOutput").ap()

    EXP = mybir.ActivationFunctionType.Exp
    MULT = mybir.AluOpType.mult
    ADD = mybir.AluOpType.add

    with tile.TileContext(nc) as tc:
        with (
            tc.tile_pool(name="win", bufs=1) as win,        # weights + x + consts
            tc.tile_pool(name="kq", bufs=1) as kqp,         # K/Q bf16 tiles
            tc.tile_pool(name="vp", bufs=1) as vp,          # [V|1] tiles
            tc.tile_pool(name="pex", bufs=6) as pex,        # exp outputs
            tc.tile_pool(name="acat", bufs=1) as acatp,     # normalized attn (d, nq)
            tc.tile_pool(name="scr", bufs=1) as scr,        # norm scratch
            tc.tile_pool(name="osb", bufs=6) as osbp,       # out eviction
            tc.tile_pool(name="ps", bufs=1, space="PSUM") as psp,
        ):
            # ---- warm-up inputs (no DMA deps; memsets on vector).  Dummies
            # use the full 128x128 array so the HAM activity monitor counts
            # them and un-throttles the PE clock before real data lands. ----
            dumw = win.tile([128, 128], bf16, tag="dumw")
            dumi = win.tile([128, 512], bf16, tag="dumi")
            nc.vector.memset(dumw[:], 0.0)
            nc.vector.memset(dumi[:], 0.0)
            # broadcast row for the denominator: M=65 so the product lands on
            # partitions 1..64, aligned with the den-at-row-0 PV layout
            ones65 = win.tile([1, 65], bf16, tag="ones65")
            nc.vector.memset(ones65[:], 1.0)

            # ---- loads, criticality-sorted across two queues ----
            # sync queue: bq, x token-half 0, wv, x token-half 1
            # gpsimd queue: wqk 256-col chunks (K/Q for pairs 0-1 first), wo
            bqt = win.tile([128, 4], f32, tag="bq")
            nc.sync.dma_start(bqt[:], bq_d[:])
            xt, wqk, wv = [], [], []
            for kc in range(KC):
                t = win.tile([128, N], bf16, tag=f"xt{kc}")
                nc.sync.dma_start(
                    t[:, 0:1024], xT_d[kc * 128:(kc + 1) * 128, 0:1024]
                )
                xt.append(t)
                t = win.tile([128, 1024], bf16, tag=f"wqk{kc}")
                wqk.append(t)
                t = win.tile([128, 512], bf16, tag=f"wv{kc}")
                wv.append(t)
            # K chunks for pairs 0-1 ([512:768]) then Q chunks pairs 0-1
            for lo in (512, 0):
                for kc in range(KC):
                    nc.gpsimd.dma_start(
                        wqk[kc][:, lo:lo + 256],
                        wqk_d[kc * 128:(kc + 1) * 128, lo:lo + 256],
                    )
            for kc in range(KC):
                nc.sync.dma_start(wv[kc][:], wv_d[kc * 128:(kc + 1) * 128, :])
            for kc in range(KC):
                nc.sync.dma_start(
                    xt[kc][:, 1024:2048], xT_d[kc * 128:(kc + 1) * 128, 1024:2048]
                )
            # K/Q chunks for pairs 2-3
            for lo in (768, 256):
                for kc in range(KC):
                    nc.gpsimd.dma_start(
                        wqk[kc][:, lo:lo + 256],
                        wqk_d[kc * 128:(kc + 1) * 128, lo:lo + 256],
                    )
            wo = []
            for dc in range(4):
                t = win.tile([128, 1024], bf16, tag=f"wo{dc}")
                nc.gpsimd.dma_start(t[:], wo_d[dc * 128:(dc + 1) * 128, :])
                wo.append(t)

            acat = [[None] * PAIRS for _ in range(NQ)]

            # shared pp/rb tag alternator: consecutive projection chains
            # land in different banks so eviction overlaps the next chain
            ps_flip = [0]

            def proj_ps():
                ps_flip[0] ^= 1
                return psp.tile(
                    [128, 512], f32, name="ps",
                    tag=("pp" if ps_flip[0] else "rb"),
                )

            # ---- PE warm-up: dummies into the pp bank, serial, data-free ----
            warm_ps = psp.tile([128, 512], f32, name="ps", tag="pp")
            for _ in range(N_WARM):
                nc.tensor.matmul(
                    warm_ps[:], dumw[:], dumi[:],
                    start=True, stop=True,
                )

            def kq_group(t, kt, qt, j):
                kind, nq = divmod(j, NQ)
                ps = proj_ps()
                off = (512 if kind == 0 else 0) + t * 128
                for kc in range(KC):
                    nc.tensor.matmul(
                        ps[:],
                        wqk[kc][:, off:off + 128],
                        xt[kc][:, nq * 512:(nq + 1) * 512],
                        start=(kc == 0), stop=(kc == KC - 1),
                    )
                if kind == 0:
                    nc.vector.tensor_copy(kt[:, nq * 512:(nq + 1) * 512], ps[:])
                else:
                    nc.vector.tensor_scalar_add(
                        qt[:, nq * 512:(nq + 1) * 512], ps[:], bqt[:, t:t + 1]
                    )

            def v_proj(mq):
                # ones column FIRST: the softmax denominator accumulates at
                # PSUM row 0, where reciprocal_approx_fast can read it
                # directly (the custom op only works at base partition 0)
                vt = vp.tile([128, 8, 65], bf16, tag=f"v{mq}")
                nc.vector.memset(vt[:, :, 0:1], 1.0)
                ps = proj_ps()
                for kc in range(KC):
                    nc.tensor.matmul(
                        ps[:], xt[kc][:, mq * 128:(mq + 1) * 128], wv[kc][:],
                        start=(kc == 0), stop=(kc == KC - 1),
                    )
                nc.vector.tensor_copy(
                    vt[:, :, 1:65], ps[:].rearrange("p (h e) -> p h e", h=8)
                )
                return vt

            vts = [None] * MQ

            def v_chain(mq):
                def go():
                    vts[mq] = v_proj(mq)
                return go

            J_ORDER = [0, 4, 1, 5, 2, 6, 3, 7]

            def out_proj_cc(nq, cc):
                ps = proj_ps()
                for dc in range(4):
                    nc.tensor.matmul(
                        ps[:],
                        wo[dc][:, cc * 128:(cc + 1) * 128],
                        acat[nq][dc][:],
                        start=(dc == 0), stop=(dc == 3),
                    )
                ob = osbp.tile([128, 512], bf16, tag="ob")
                nc.vector.tensor_copy(ob[:], ps[:])
                # last chunk's stores go to the idle sync queue so the
                # gpsimd queue's end-of-kernel drain doesn't serialize them
                eng = nc.sync if nq == NQ - 1 else nc.gpsimd
                eng.dma_start(
                    outT_d[cc * 128:(cc + 1) * 128, nq * 512:(nq + 1) * 512],
                    ob[:],
                )

            # Deferred normalization: stage 0 (evict + reciprocal) runs at the
            # nq seam; the remaining stages are emitted one-per-mq inside the
            # next nq's loop so their PE work never waits on DVE round-trips.
            def make_norm(t, nq, atA, atB):
                ac = acatp.tile([128, 512], bf16, tag=f"ac{nq}_{t}")
                acat[nq][t] = ac
                st = {}

                def stage0():
                    sA = scr.tile([65, 512], f32, tag="sA")
                    sB = scr.tile([65, 512], f32, tag="sB")
                    r32A = scr.tile([1, 512], f32, tag="r32A")
                    r32B = scr.tile([1, 512], f32, tag="r32B")
                    # jump the DVE queue: the reciprocals (reading the PSUM
                    # den row 0 directly) start the den critical path with
                    # no eviction/DMA hop; the evictions gate the next nq's
                    # first PV (PSUM WAR) and with it the whole PE pipe
                    with tc.high_priority():
                        nc.vector.reciprocal_approx_fast(
                            r32A[0:1, :], atA[0:1, :]
                        )
                        nc.vector.reciprocal_approx_fast(
                            r32B[0:1, :], atB[0:1, :]
                        )
                        # full-range evictions: PSUM APs must start at a
                        # 32-aligned partition, so include the den row
                        nc.vector.tensor_copy(sA[:], atA[:])
                        nc.vector.tensor_copy(sB[:], atB[:])
                    rrA = scr.tile([1, 512], bf16, tag="rrA")
                    rrB = scr.tile([1, 512], bf16, tag="rrB")
                    nc.vector.tensor_copy(rrA[0:1, :], r32A[0:1, :])
                    nc.vector.tensor_copy(rrB[0:1, :], r32B[0:1, :])
                    st.update(sA=sA, sB=sB, rrA=rrA, rrB=rrB)

                def half(which):
                    s, rr = (st["sA"], st["rrA"]) if which == 0 else (st["sB"], st["rrB"])
                    rb = psp.tile([65, 512], f32, tag="rb")
                    nc.tensor.matmul(
                        rb[:], ones65[0:1, :], rr[0:1, :],
                        start=True, stop=True,
                    )
                    acn = scr.tile([65, 512], bf16, tag=f"acn{which}")
                    nc.vector.tensor_mul(acn[:], s[:], rb[:])
                    nc.sync.dma_start(
                        ac[which * 64:(which + 1) * 64, :], acn[1:65, :]
                    )

                return [stage0, lambda: half(0), lambda: half(1)] + (
                    [lambda cc=cc: out_proj_cc(nq, cc) for cc in range(8)]
                    if t == PAIRS - 1 else []
                )

            pending: list = []
            deferred: list = []   # pair-0 local chains, drained 1/mq-slot
            prefetch: list = []   # next-pair chains, drained every few slots

            # ---- pair 0 minimal pre-loop: K(nq0), Q(nq0), V(0..3) ----
            kt0 = kqp.tile([128, N], bf16, tag="k0")
            qt0 = kqp.tile([128, N], bf16, tag="q0")
            kq_group(0, kt0, qt0, 0)
            kq_group(0, kt0, qt0, 4)
            for mq in range(4):
                vts[mq] = v_proj(mq)
            # remaining pair-0 work, deadline-ordered for the mq-slot drain
            deferred += [lambda: kq_group(0, kt0, qt0, 1)]
            deferred += [v_chain(m) for m in (4, 5, 6)]
            deferred += [lambda: kq_group(0, kt0, qt0, 2)]
            deferred += [v_chain(m) for m in (7, 8, 9)]
            deferred += [lambda: kq_group(0, kt0, qt0, 3)]
            deferred += [v_chain(m) for m in (10, 11, 12)]
            deferred += [lambda: kq_group(0, kt0, qt0, 5)]
            deferred += [v_chain(m) for m in (13, 14, 15)]
            deferred += [
                lambda: kq_group(0, kt0, qt0, 6),
                lambda: kq_group(0, kt0, qt0, 7),
            ]
            kq_tiles = {0: (kt0, qt0)}

            # ---- per pair: attention; drain deferred chains; pipelined norm ----
            for t in range(PAIRS):
                kt, qt = kq_tiles.pop(t)
                if t + 1 < PAIRS:
                    ktn = kqp.tile([128, N], bf16, tag=f"k{(t + 1) % 2}")
                    qtn = kqp.tile([128, N], bf16, tag=f"q{(t + 1) % 2}")
                    kq_tiles[t + 1] = (ktn, qtn)
                    # only the chains pair t+1 needs at its nq0 prefetch
                    # during pair t; its Q chains for nq1-3 drain during its
                    # own early slots (relieves the PE-bound pair-0 windows)
                    for j in (0, 4, 1, 2, 3):
                        prefetch.append(
                            lambda t1=t + 1, kt1=ktn, qt1=qtn, j1=j:
                            kq_group(t1, kt1, qt1, j1)
                        )
                if t > 0:
                    deferred += [
                        lambda j1=j: kq_group(t, kt, qt, j1)
                        for j in (5, 6, 7)
                    ]
                # next-pair chains spread over this pair's ScalarE-bound
                # windows: pair 0 over nq 1-3, others over all 4
                nq_lo, step = (1, 5) if t == 0 else (0, 8)
                for nq in range(NQ):
                    atA = psp.tile([65, 512], f32, tag="atA")
                    atB = psp.tile([65, 512], f32, tag="atB")
                    for mq in range(MQ):
                        if deferred:
                            deferred.pop(0)()
                        elif prefetch and nq >= nq_lo:
                            fi = (nq - nq_lo) * MQ + mq
                            if fi % step == 0:
                                prefetch.pop(0)()
                        sp = psp.tile([128, 1024], f32, tag="sp", bufs=2)
                        # highest priority: S results feed ScalarE, the
                        # second rail — never let exp starve behind proj work
                        with tc.high_priority():
                            nc.tensor.matmul(
                                sp[:, 0:512],
                                kt[0:64, mq * 128:(mq + 1) * 128],
                                qt[0:64, nq * 512:(nq + 1) * 512],
                                start=True, stop=True, tile_position=(0, 0),
                            )
                            nc.tensor.matmul(
                                sp[:, 512:1024],
                                kt[64:128, mq * 128:(mq + 1) * 128],
                                qt[64:128, nq * 512:(nq + 1) * 512],
                                start=True, stop=True, tile_position=(64, 0),
                            )
                        # drain one stage of the deferred norm pipeline per mq
                        if pending and 1 <= mq <= len(pending):
                            pending[mq - 1]()
                            if mq == len(pending):
                                pending.clear()
                        # deep pe ring: ACT(k) only WARs on PV(k-12), so the
                        # exp stream never waits on V-projection availability
                        # in (0,0) and rides over every nq seam
                        pe = pex.tile([128, 1024], bf16, tag="pe", bufs=16)
                        if mq in SCHR_MQS[t]:
                            # Schraudolph exp on DVE: offloads the ScalarE rail
                            nc.vector.tensor_scalar(
                                pe[:].bitcast(i16), sp[:],
                                SCHR_SCALE, SCHR_MAGIC, MULT, ADD,
                            )
                        else:
                            nc.scalar.activation(pe[:], sp[:], EXP, scale=ATTN_SCALE)
                        nc.tensor.matmul(
                            atA[:], vts[mq][:, 2 * t, :], pe[:, 0:512],
                            start=(mq == 0), stop=(mq == MQ - 1),
                        )
                        nc.tensor.matmul(
                            atB[:], vts[mq][:, 2 * t + 1, :], pe[:, 512:1024],
                            start=(mq == 0), stop=(mq == MQ - 1),
                        )
                    stages = make_norm(t, nq, atA, atB)
                    # evict + reciprocal immediately (clears PSUM); defer rest
                    stages[0]()
                    pending = stages[1:]
            # tail warm-keepers: cover the final norm latency so the HAM
            # clock-gate stays at 8/8 for the last out-projection chains
            for _ in range(20):
                wps = psp.tile([65, 512], f32, name="ps", tag="atB")
                nc.tensor.matmul(
                    wps[:], dumw[:, 0:65], dumi[:], start=True, stop=True,
                )
            # drain the last norm (pair 3, nq 3) and its out projection
            for s in pending:
                s()
            pending = []

    nc.compile()
    return nc


def _get_program():
    if "nc" not in _cache:
        _cache["nc"] = _build_program()
    return _cache["nc"]


def _prep_in_maps(x, W_qkv, W_lora, b_lora, A_q, B_q, A_v, B_v, W_out):
    HD = H * D  # 1024
    Wq = W_qkv[0:HD] + W_lora[0:HD] + LORA_SCALE * (B_q @ A_q)
    Wk = W_qkv[HD:2 * HD]
    Wv = W_qkv[2 * HD:3 * HD] + W_lora[2 * HD:3 * HD] + LORA_SCALE * (B_v @ A_v)
    bq = b_lora[0:HD]

    xT = [np.ascontiguousarray(x[b].T).astype(BF) for b in range(B)]
    in_maps = []
    for c in range(8):
        b, hg = divmod(c, 2)
        sel = slice(hg * 512, (hg + 1) * 512)
        wqk_c = np.ascontiguousarray(
            np.concatenate([Wq[sel], Wk[sel]], axis=0).T
        ).astype(BF)
        wv_c = np.ascontiguousarray(Wv[sel].T).astype(BF)
        wo_c = np.ascontiguousarray(W_out[:, sel].T).astype(BF)
        bq_c = np.ascontiguousarray(bq[sel].reshape(4, 128).T).astype(np.float32)
        in_maps.append({
            "xT": xT[b], "wqk": wqk_c, "wv": wv_c, "wo": wo_c, "bq": bq_c,
        })
    return in_maps


def kernel(x, W_qkv, W_lora, b_lora, A_q, B_q, A_v, B_v, W_out, b_out):
    x = np.asarray(x, np.float32)
    W_qkv = np.asarray(W_qkv, np.float32)
    W_lora = np.asarray(W_lora, np.float32)
    b_lora = np.asarray(b_lora, np.float32)
    A_q = np.asarray(A_q, np.float32)
    B_q = np.asarray(B_q, np.float32)
    A_v = np.asarray(A_v, np.float32)
    B_v = np.asarray(B_v, np.float32)
    W_out = np.asarray(W_out, np.float32)
    b_out = np.asarray(b_out, np.float32)

    in_maps = _prep_in_maps(x, W_qkv, W_lora, b_lora, A_q, B_q, A_v, B_v, W_out)
    b_eff = b_out + W_out @ b_lora[2 * H * D:3 * H * D]

    nc = _get_program()
    res = run_bass_kernel_spmd(nc, in_maps, list(range(8)))

    out = np.empty((B, N, C), np.float32)
    for b in range(B):
        acc = res.results[2 * b]["outT"].astype(np.float32)
        acc += res.results[2 * b + 1]["outT"].astype(np.float32)
        acc += b_eff[:, None]
        out[b] = acc.T
    return out


# revision 27
# speedup vs baseline: 1.0633x; 1.0015x over previous
"""Trainium2 Bass kernel for nn_LoraAttention.

Math (reference): qkv = x@W_qkv.T; lora full proj ql/vl = split(x@W_lora.T + b_lora)
(K-part discarded); low-rank dq = (x@A_q.T)@B_q.T*1/8 (same for v); softmax
attention over H=16 heads, D=64; out = attn_cat@W_out.T + b_out.

Host-side algebra folds every LoRA term into the projection weights:
  Wq_eff = W_qkv[q] + W_lora[q] + (B_q@A_q)/8      (q bias b_lora[q] kept)
  Wk_eff = W_qkv[k]                                 (no bias)
  Wv_eff = W_qkv[v] + W_lora[v] + (B_v@A_v)/8
  v bias b_lora[v] commutes through softmax -> folded into host-side output
  bias: b_eff = b_out + W_out @ b_lora[v].

Sharding: 8 cores = 4 batches x 2 head-groups (8 heads each).  Each core
projects QKV for its heads, does attention, and computes a partial output
projection over its 512 concat dims; host sums the two partials per batch
(partials shipped bf16, summed fp32 on host).

Device dataflow per core (matmuls bf16 in / fp32 accum):
  - warm-up: ~24 dummy matmuls on memset tiles issue from t~0 so the PE HAM
    clock-gate reaches 8/8 (2.4 GHz) before real data lands.
  - DMA order is criticality-sorted and split across two queues (sync: x
    token-half 0, wv, x token-half 1; gpsimd: wqk in 256-col chunks with the
    pair-0/1 K and Q chunks first, then wo) so the first S^T can issue ~17us.
  - pair-0's V projections and remaining K/Q chains drain one-per-mq-slot
    inside pair 0's attention windows instead of running as a monolithic
    block that starves ScalarE.
  - S^T = K^T@Q per head via row-packed (tile_position) pairs of K=64
    matmuls; exp on ScalarE from PSUM (scale=1/8, bf16 out); P@[V|1] matmuls
    put raw attention in rows 0..63 and the softmax denominator in row 64.
  - a subset of key-chunks (3 of 16 per window, skipping pair0-nq0) computes
    exp on the DVE instead via the Schraudolph bit trick: one tensor_scalar
    (s*23.083+16250.5 -> int16, bitcast bf16) approximates exp(s/8) to ~3%;
    the constant offset cancels in softmax normalization.  This moves ~18%
    of the exp rail off the saturated ScalarE.
  - normalization per (pair, nq), software-pipelined by one nq so nothing
    stalls the in-order PE queue (deferred-stage machinery as before).
  - output projection for chunk nq emitted inside pair 3 right after that
    nq's normalization; partials evicted bf16 and DMA'd from the gpsimd
    queue.
"""

import numpy as np
import ml_dtypes

import concourse.bacc as bacc
import concourse.tile as tile
from concourse import mybir
from concourse.bass_utils import run_bass_kernel_spmd

B, N, C = 4, 2048, 1024
H, D = 16, 64
LORA_SCALE = 1.0 / 8.0
ATTN_SCALE = float(D) ** -0.5  # 0.125

f32 = mybir.dt.float32
bf16 = mybir.dt.bfloat16
i16 = mybir.dt.int16
BF = ml_dtypes.bfloat16

NQ = 4           # token chunks of 512 for moving operands
MQ = 16          # key/token chunks of 128 for S^T partition dim
KC = 8           # contraction chunks of 128 over C
PAIRS = 4        # head pairs per core (8 local heads)

N_WARM = 44      # PE warm-up dummy matmuls (bridge until DMA-fed chains start)

# Schraudolph exp-approx constants: exp(s*0.125) ~= bitcast_bf16(int16(
#   s * (0.125*128/ln2) + (16256 - 128*0.043) )).  The -0.043 shift centres
# the piecewise-linear 2^frac error; any constant offset cancels in softmax.
SCHR_SCALE = 0.125 * 128.0 / float(np.log(2.0))
SCHR_MAGIC = 16256.0 - 128.0 * 0.043
# Per-pair sets of mq slots whose exp tile is computed on the DVE instead of
# ScalarE.  Few per window: the sp ring couples S^T(k+2) to reader(k), and
# the DVE (shared with evictions/norm work) serves reads less promptly than
# the dedicated ACT queue — more offload measurably slows the ring.  Pair 0
# is PE-bound (chain deficit) and pair 3's DVE carries out-proj evictions.
# All offloaded slots share ONE parity (even mq): the two sp buffers
# alternate by slot parity, forming two independent reader chains.  With
# even slots read by the DVE, the odd chain paces at back-to-back ACT-exp
# speed while the even chain free-runs — the window becomes PE-bound
# instead of locking every slot to the 1113ns exp duration.
SCHR_MQS = {
    0: (),
    1: (2, 4, 6, 10, 12, 14),
    2: (2, 4, 6, 10, 12, 14),
    3: (2, 6, 10, 14),
}

_cache: dict = {}


def _build_program():
    nc = bacc.Bacc("TRN2", target_bir_lowering=False, debug=False, num_devices=8)

    xT_d = nc.dram_tensor("xT", [C, N], bf16, kind="ExternalInput").ap()
    wqk_d = nc.dram_tensor("wqk", [C, 1024], bf16, kind="ExternalInput").ap()
    wv_d = nc.dram_tensor("wv", [C, 512], bf16, kind="ExternalInput").ap()
    wo_d = nc.dram_tensor("wo", [512, C], bf16, kind="ExternalInput").ap()
    bq_d = nc.dram_tensor("bq", [128, 4], f32, kind="ExternalInput").ap()
    outT_d = nc.dram_tensor("outT", [C, N], bf16, kind="ExternalOutput").ap()

    EXP = mybir.ActivationFunctionType.Exp
    MULT = mybir.AluOpType.mult
    ADD = mybir.AluOpType.add

    with tile.TileContext(nc) as tc:
        with (
            tc.tile_pool(name="win", bufs=1) as win,        # weights + x + consts
            tc.tile_pool(name="kq", bufs=1) as kqp,         # K/Q bf16 tiles
            tc.tile_pool(name="vp", bufs=1) as vp,          # [V|1] tiles
            tc.tile_pool(name="pex", bufs=6) as pex,        # exp outputs
            tc.tile_pool(name="acat", bufs=1) as acatp,     # normalized attn (d, nq)
            tc.tile_pool(name="scr", bufs=1) as scr,        # norm scratch
            tc.tile_pool(name="osb", bufs=6) as osbp,       # out eviction
            tc.tile_pool(name="ps", bufs=1, space="PSUM") as psp,
        ):
            # ---- warm-up inputs (no DMA deps; memsets on vector).  Dummies
            # use the full 128x128 array so the HAM activity monitor counts
            # them and un-throttles the PE clock before real data lands. ----
            dumw = win.tile([128, 128], bf16, tag="dumw")
            dumi = win.tile([128, 512], bf16, tag="dumi")
            nc.vector.memset(dumw[:], 0.0)
            nc.vector.memset(dumi[:], 0.0)
            # broadcast row for the denominator: M=65 so the product lands on
            # partitions 1..64, aligned with the den-at-row-0 PV layout
            ones65 = win.tile([1, 65], bf16, tag="ones65")
            nc.vector.memset(ones65[:], 1.0)

            # ---- loads, criticality-sorted across two queues ----
            # sync queue: bq, x token-half 0, wv, x token-half 1
            # gpsimd queue: wqk 256-col chunks (K/Q for pairs 0-1 first), wo
            bqt = win.tile([128, 4], f32, tag="bq")
            nc.sync.dma_start(bqt[:], bq_d[:])
            xt, wqk, wv = [], [], []
            for kc in range(KC):
                t = win.tile([128, N], bf16, tag=f"xt{kc}")
                nc.sync.dma_start(
                    t[:, 0:1024], xT_d[kc * 128:(kc + 1) * 128, 0:1024]
                )
                xt.append(t)
                t = win.tile([128, 1024], bf16, tag=f"wqk{kc}")
                wqk.append(t)
                t = win.tile([128, 512], bf16, tag=f"wv{kc}")
                wv.append(t)
            # K chunks for pairs 0-1 ([512:768]) then Q chunks pairs 0-1
            for lo in (512, 0):
                for kc in range(KC):
                    nc.gpsimd.dma_start(
                        wqk[kc][:, lo:lo + 256],
                        wqk_d[kc * 128:(kc + 1) * 128, lo:lo + 256],
                    )
            for kc in range(KC):
                nc.sync.dma_start(wv[kc][:], wv_d[kc * 128:(kc + 1) * 128, :])
            for kc in range(KC):
                nc.sync.dma_start(
                    xt[kc][:, 1024:2048], xT_d[kc * 128:(kc + 1) * 128, 1024:2048]
                )
            # K/Q chunks for pairs 2-3
            for lo in (768, 256):
                for kc in range(KC):
                    nc.gpsimd.dma_start(
                        wqk[kc][:, lo:lo + 256],
                        wqk_d[kc * 128:(kc + 1) * 128, lo:lo + 256],
                    )
            wo = []
            for dc in range(4):
                t = win.tile([128, 1024], bf16, tag=f"wo{dc}")
                nc.gpsimd.dma_start(t[:], wo_d[dc * 128:(dc + 1) * 128, :])
                wo.append(t)

            acat = [[None] * PAIRS for _ in range(NQ)]

            # shared pp/rb tag alternator: consecutive projection chains
            # land in different banks so eviction overlaps the next chain
            ps_flip = [0]

            def proj_ps():
                ps_flip[0] ^= 1
                return psp.tile(
                    [128, 512], f32, name="ps",
                    tag=("pp" if ps_flip[0] else "rb"),
                )

            # ---- PE warm-up: dummies into the pp bank, serial, data-free ----
            warm_ps = psp.tile([128, 512], f32, name="ps", tag="pp")
            for _ in range(N_WARM):
                nc.tensor.matmul(
                    warm_ps[:], dumw[:], dumi[:],
                    start=True, stop=True,
                )

            def kq_group(t, kt, qt, j):
                kind, nq = divmod(j, NQ)
                ps = proj_ps()
                off = (512 if kind == 0 else 0) + t * 128
                for kc in range(KC):
                    nc.tensor.matmul(
                        ps[:],
                        wqk[kc][:, off:off + 128],
                        xt[kc][:, nq * 512:(nq + 1) * 512],
                        start=(kc == 0), stop=(kc == KC - 1),
                    )
                if kind == 0:
                    nc.vector.tensor_copy(kt[:, nq * 512:(nq + 1) * 512], ps[:])
                else:
                    nc.vector.tensor_scalar_add(
                        qt[:, nq * 512:(nq + 1) * 512], ps[:], bqt[:, t:t + 1]
                    )

            def v_proj(mq):
                # ones column FIRST: the softmax denominator accumulates at
                # PSUM row 0, where reciprocal_approx_fast can read it
                # directly (the custom op only works at base partition 0)
                vt = vp.tile([128, 8, 65], bf16, tag=f"v{mq}")
                nc.vector.memset(vt[:, :, 0:1], 1.0)
                ps = proj_ps()
                for kc in range(KC):
                    nc.tensor.matmul(
                        ps[:], xt[kc][:, mq * 128:(mq + 1) * 128], wv[kc][:],
                        start=(kc == 0), stop=(kc == KC - 1),
                    )
                nc.vector.tensor_copy(
                    vt[:, :, 1:65], ps[:].rearrange("p (h e) -> p h e", h=8)
                )
                return vt

            vts = [None] * MQ

            def v_chain(mq):
                def go():
                    vts[mq] = v_proj(mq)
                return go

            J_ORDER = [0, 4, 1, 5, 2, 6, 3, 7]

            def out_proj_cc(nq, cc):
                ps = proj_ps()
                for dc in range(4):
                    nc.tensor.matmul(
                        ps[:],
                        wo[dc][:, cc * 128:(cc + 1) * 128],
                        acat[nq][dc][:],
                        start=(dc == 0), stop=(dc == 3),
                    )
                ob = osbp.tile([128, 512], bf16, tag="ob")
                nc.vector.tensor_copy(ob[:], ps[:])
                # last chunk's stores go to the idle sync queue so the
                # gpsimd queue's end-of-kernel drain doesn't serialize them
                eng = nc.sync if nq == NQ - 1 else nc.gpsimd
                eng.dma_start(
                    outT_d[cc * 128:(cc + 1) * 128, nq * 512:(nq + 1) * 512],
                    ob[:],
                )

            # Deferred normalization: stage 0 (evict + reciprocal) runs at the
            # nq seam; the remaining stages are emitted one-per-mq inside the
            # next nq's loop so their PE work never waits on DVE round-trips.
            def make_norm(t, nq, atA, atB):
                ac = acatp.tile([128, 512], bf16, tag=f"ac{nq}_{t}")
                acat[nq][t] = ac
                st = {}

                def stage0():
                    sA = scr.tile([65, 512], f32, tag="sA")
                    sB = scr.tile([65, 512], f32, tag="sB")
                    r32A = scr.tile([1, 512], f32, tag="r32A")
                    r32B = scr.tile([1, 512], f32, tag="r32B")
                    # jump the DVE queue: the reciprocals (reading the PSUM
                    # den row 0 directly) start the den critical path with
                    # no eviction/DMA hop; the evictions gate the next nq's
                    # first PV (PSUM WAR) and with it the whole PE pipe
                    with tc.high_priority():
                        nc.vector.reciprocal_approx_fast(
                            r32A[0:1, :], atA[0:1, :]
                        )
                        nc.vector.reciprocal_approx_fast(
                            r32B[0:1, :], atB[0:1, :]
                        )
                        # full-range evictions: PSUM APs must start at a
                        # 32-aligned partition, so include the den row
                        nc.vector.tensor_copy(sA[:], atA[:])
                        nc.vector.tensor_copy(sB[:], atB[:])
                    rrA = scr.tile([1, 512], bf16, tag="rrA")
                    rrB = scr.tile([1, 512], bf16, tag="rrB")
                    nc.vector.tensor_copy(rrA[0:1, :], r32A[0:1, :])
                    nc.vector.tensor_copy(rrB[0:1, :], r32B[0:1, :])
                    st.update(sA=sA, sB=sB, rrA=rrA, rrB=rrB)

                def half(which):
                    s, rr = (st["sA"], st["rrA"]) if which == 0 else (st["sB"], st["rrB"])
                    rb = psp.tile([65, 512], f32, tag="rb")
                    nc.tensor.matmul(
                        rb[:], ones65[0:1, :], rr[0:1, :],
                        start=True, stop=True,
                    )
                    acn = scr.tile([65, 512], bf16, tag=f"acn{which}")
                    nc.vector.tensor_mul(acn[:], s[:], rb[:])
                    nc.sync.dma_start(
                        ac[which * 64:(which + 1) * 64, :], acn[1:65, :]
                    )

                return [stage0, lambda: half(0), lambda: half(1)] + (
                    [lambda cc=cc: out_proj_cc(nq, cc) for cc in range(8)]
                    if t == PAIRS - 1 else []
                )

            pending: list = []
            deferred: list = []   # pair-0 local chains, drained 1/mq-slot
            prefetch: list = []   # next-pair chains, drained every few slots

            # ---- pair 0 minimal pre-loop: K(nq0), Q(nq0), V(0..3) ----
            kt0 = kqp.tile([128, N], bf16, tag="k0")
            qt0 = kqp.tile([128, N], bf16, tag="q0")
            kq_group(0, kt0, qt0, 0)
            kq_group(0, kt0, qt0, 4)
            for mq in range(4):
                vts[mq] = v_proj(mq)
            # remaining pair-0 work, deadline-ordered for the mq-slot drain
            deferred += [lambda: kq_group(0, kt0, qt0, 1)]
            deferred += [v_chain(m) for m in (4, 5, 6)]
            deferred += [lambda: kq_group(0, kt0, qt0, 2)]
            deferred += [v_chain(m) for m in (7, 8, 9)]
            deferred += [lambda: kq_group(0, kt0, qt0, 3)]
            deferred += [v_chain(m) for m in (10, 11, 12)]
            deferred += [lambda: kq_group(0, kt0, qt0, 5)]
            deferred += [v_chain(m) for m in (13, 14, 15)]
            deferred += [
                lambda: kq_group(0, kt0, qt0, 6),
                lambda: kq_group(0, kt0, qt0, 7),
            ]
            kq_tiles = {0: (kt0, qt0)}

            # ---- per pair: attention; drain deferred chains; pipelined norm ----
            for t in range(PAIRS):
                kt, qt = kq_tiles.pop(t)
                if t + 1 < PAIRS:
                    ktn = kqp.tile([128, N], bf16, tag=f"k{(t + 1) % 2}")
                    qtn = kqp.tile([128, N], bf16, tag=f"q{(t + 1) % 2}")
                    kq_tiles[t + 1] = (ktn, qtn)
                    # only the chains pair t+1 needs at its nq0 prefetch
                    # during pair t; its Q chains for nq1-3 drain during its
                    # own early slots (relieves the PE-bound pair-0 windows)
                    for j in (0, 4, 1, 2, 3):
                        prefetch.append(
                            lambda t1=t + 1, kt1=ktn, qt1=qtn, j1=j:
                            kq_group(t1, kt1, qt1, j1)
                        )
                if t > 0:
                    deferred += [
                        lambda j1=j: kq_group(t, kt, qt, j1)
                        for j in (5, 6, 7)
                    ]
                # next-pair chains spread over this pair's ScalarE-bound
                # windows: pair 0 over nq 1-3, others over all 4
                nq_lo, step = (1, 5) if t == 0 else (0, 8)
                # pair-0's catch-up chains drain 1/slot; later pairs spread
                # their chains every 8th slot so pair boundaries don't bunch
                # ~5us of projection work in front of the exp stream
                own_step = 1 if t == 0 else 8
                for nq in range(NQ):
                    atA = psp.tile([65, 512], f32, tag="atA")
                    atB = psp.tile([65, 512], f32, tag="atB")
                    for mq in range(MQ):
                        si = nq * MQ + mq
                        if deferred and si % own_step == 0:
                            deferred.pop(0)()
                        elif prefetch and nq >= nq_lo:
                            fi = (nq - nq_lo) * MQ + mq
                            if fi % step == 0:
                                prefetch.pop(0)()
                        sp = psp.tile([128, 1024], f32, tag="sp", bufs=2)
                        # highest priority: S results feed ScalarE, the
                        # second rail — never let exp starve behind proj work
                        with tc.high_priority():
                            nc.tensor.matmul(
                                sp[:, 0:512],
                                kt[0:64, mq * 128:(mq + 1) * 128],
                                qt[0:64, nq * 512:(nq + 1) * 512],
                                start=True, stop=True, tile_position=(0, 0),
                            )
                            nc.tensor.matmul(
                                sp[:, 512:1024],
                                kt[64:128, mq * 128:(mq + 1) * 128],
                                qt[64:128, nq * 512:(nq + 1) * 512],
                                start=True, stop=True, tile_position=(64, 0),
                            )
                        # drain one stage of the deferred norm pipeline per mq
                        if pending and 1 <= mq <= len(pending):
                            pending[mq - 1]()
                            if mq == len(pending):
                                pending.clear()
                        # deep pe ring: ACT(k) only WARs on PV(k-12), so the
                        # exp stream never waits on V-projection availability
                        # in (0,0) and rides over every nq seam
                        pe = pex.tile([128, 1024], bf16, tag="pe", bufs=16)
                        if mq in SCHR_MQS[t]:
                            # Schraudolph exp on DVE: offloads the ScalarE rail
                            nc.vector.tensor_scalar(
                                pe[:].bitcast(i16), sp[:],
                                SCHR_SCALE, SCHR_MAGIC, MULT, ADD,
                            )
                        else:
                            nc.scalar.activation(pe[:], sp[:], EXP, scale=ATTN_SCALE)
                        nc.tensor.matmul(
                            atA[:], vts[mq][:, 2 * t, :], pe[:, 0:512],
                            start=(mq == 0), stop=(mq == MQ - 1),
                        )
                        nc.tensor.matmul(
                            atB[:], vts[mq][:, 2 * t + 1, :], pe[:, 512:1024],
                            start=(mq == 0), stop=(mq == MQ - 1),
                        )
                    stages = make_norm(t, nq, atA, atB)
                    # evict + reciprocal immediately (clears PSUM); defer rest
                    stages[0]()
                    pending = stages[1:]
            # tail warm-keepers: cover the final norm latency so the HAM
            # clock-gate stays at 8/8 for the last out-projection chains
            for _ in range(12):
                wps = psp.tile([65, 512], f32, name="ps", tag="atB")
                nc.tensor.matmul(
                    wps[:], dumw[:, 0:65], dumi[:], start=True, stop=True,
                )
            # drain the last norm (pair 3, nq 3) and its out projection
            for s in pending:
                s()
            pending = []

    nc.compile()
    return nc


def _get_program():
    if "nc" not in _cache:
        _cache["nc"] = _build_program()
    return _cache["nc"]


def _prep_in_maps(x, W_qkv, W_lora, b_lora, A_q, B_q, A_v, B_v, W_out):
    HD = H * D  # 1024
    Wq = W_qkv[0:HD] + W_lora[0:HD] + LORA_SCALE * (B_q @ A_q)
    Wk = W_qkv[HD:2 * HD]
    Wv = W_qkv[2 * HD:3 * HD] + W_lora[2 * HD:3 * HD] + LORA_SCALE * (B_v @ A_v)
    bq = b_lora[0:HD]

    xT = [np.ascontiguousarray(x[b].T).astype(BF) for b in range(B)]
    in_maps = []
    for c in range(8):
        b, hg = divmod(c, 2)
        sel = slice(hg * 512, (hg + 1) * 512)
        wqk_c = np.ascontiguousarray(
            np.concatenate([Wq[sel], Wk[sel]], axis=0).T
        ).astype(BF)
        wv_c = np.ascontiguousarray(Wv[sel].T).astype(BF)
        wo_c = np.ascontiguousarray(W_out[:, sel].T).astype(BF)
        bq_c = np.ascontiguousarray(bq[sel].reshape(4, 128).T).astype(np.float32)
        in_maps.append({
            "xT": xT[b], "wqk": wqk_c, "wv": wv_c, "wo": wo_c, "bq": bq_c,
        })
    return in_maps


def kernel(x, W_qkv, W_lora, b_lora, A_q, B_q, A_v, B_v, W_out, b_out):
    x = np.asarray(x, np.float32)
    W_qkv = np.asarray(W_qkv, np.float32)
    W_lora = np.asarray(W_lora, np.float32)
    b_lora = np.asarray(b_lora, np.float32)
    A_q = np.asarray(A_q, np.float32)
    B_q = np.asarray(B_q, np.float32)
    A_v = np.asarray(A_v, np.float32)
    B_v = np.asarray(B_v, np.float32)
    W_out = np.asarray(W_out, np.float32)
    b_out = np.asarray(b_out, np.float32)

    in_maps = _prep_in_maps(x, W_qkv, W_lora, b_lora, A_q, B_q, A_v, B_v, W_out)
    b_eff = b_out + W_out @ b_lora[2 * H * D:3 * H * D]

    nc = _get_program()
    res = run_bass_kernel_spmd(nc, in_maps, list(range(8)))

    out = np.empty((B, N, C), np.float32)
    for b in range(B):
        acc = res.results[2 * b]["outT"].astype(np.float32)
        acc += res.results[2 * b + 1]["outT"].astype(np.float32)
        acc += b_eff[:, None]
        out[b] = acc.T
    return out
